# revision 80
# baseline (speedup 1.0000x reference)
"""Trainium2 Bass kernel for nn_Attention_50216757625003.

GQA attention layer: B=2, S=1024, D=4096, H=32 q-heads, KV=8 kv-heads,
hd=128, A=10 gated adapter tokens, RoPE, split softmax (adapter block
softmaxed separately and scaled by tanh(gate)), causal mask.

Sharding (8 NeuronCores): outer data-parallel over batch (2) x
tensor-parallel over heads (4 groups of 8 q-heads / 2 kv-heads).
wq/wk/wv are sharded column-wise, wo row-wise; each core computes
partial [S, D] output contributions (two half-head passes) and the host
sums the partials per batch element.

v2 design (causal fast path, ~339us vs the 433us f32r baseline):
  * fp16 operands everywhere: same 1 cycle/row PE rate as bf16 but 8x
    tighter rounding, half the DMA bytes of f32, and it unlocks the
    2x/4x DVE fast modes for packed 2-byte SBUF operands.
  * x is fed transposed ([D, S]); V is projected first, accumulating
    straight into all 8 psum banks (no psum->sbuf->psum round trip),
    then K and Q heads stream through 2 rotating psum banks with the
    two 512-token blocks emitted back-to-back so each bank's drain hides
    under the next block.
  * RoPE: q/k head dims are host-permuted even-first so rotation pairs
    sit on partitions p/p+64; an SBUF->SBUF DMA swaps the halves and
    four fp16 DVE ops apply the duplicated cos/sin tables.
  * scores are built transposed ([keys, q]) in 2-chunk-batched psum
    tiles ([128,1024]) so one Activation exp covers two key chunks.
    Probs stay fp16 and feed the PV matmuls directly.
  * softmax denominators never touch the PE: prob chunks are summed on
    DVE (2x fp16 tensor ops), partition-reduced+broadcast by the gpsimd
    attn-library partition_all_reduce (idle Pool engine), reciprocal'd
    on DVE, and applied as one fused multiply into the output tile.
    The adapter K/V projections (tiny, input-only) are computed on the
    host in f32 -- tanh(gate) pre-folded into the shipped adapter-V --
    and the adapter block rides the same psum accumulation, scaled by
    dt/da so the final 1/dt multiply leaves it normalized by its own
    denominator.
  * attention is software-pipelined: each (head, block) emits scores/
    exp/mask/denominator work, and its normalization tail is emitted
    one step later so the next block's mask ops lead the in-order DVE
    queue; heads interleave with the remaining Q projections.
  * the output projection runs as two half-head passes so its matmuls
    backfill the PE while the last heads' softmax chains drain; output
    DMAs are pair-batched ([128,1024] per DMA) because each dma_start
    costs the issuing sequencer ~650ns+.
  * KMM env still switches "fp16"/"bf16"/"f32r"/"f32" (non-causal masks
    fall back to the legacy general-mask program).
"""

import os
import sys

import numpy as np

for _p in ("/opt/trn_rl_repo",):
    if _p not in sys.path and os.path.isdir(_p):
        sys.path.insert(0, _p)

import concourse.bass as bass
import concourse.bass_isa as bass_isa
import concourse.mybir as mybir
from concourse import bacc
from concourse import library_config
import concourse.tile as tile
from concourse.bass_utils import run_bass_kernel_spmd

HD = 128  # head dim (hardcoded: rope split + tile shapes assume 128)
A = 10    # adapter tokens
F32 = mybir.dt.float32

MM_MODE = os.environ.get("KMM", "fp16")

_PROG_CACHE = {}


def _md(mm):
    return {"f32r": mybir.dt.float32r, "f32": mybir.dt.float32,
            "bf16": mybir.dt.bfloat16, "fp16": mybir.dt.float16}[mm]


# --------------------------------------------------------------------------
# device program (v2, causal-only fast path)
# --------------------------------------------------------------------------

def build_program_v2(KO, S, HL, KVL, mm):
    """Pipelined causal program.

    Engine split per (head, q-block): PE does scores/PV/projection matmuls
    only; Activation does exps (batched over psum pairs); DVE accumulates
    the softmax denominator from prob chunks and applies the reciprocal;
    the gpsimd (Pool) engine broadcasts partition sums via
    partition_all_reduce (attn ucode library), eliminating the baseline's
    denominator/broadcast matmuls and psum->sbuf copies.
    """
    nc = bacc.Bacc(None, target_bir_lowering=False,
                   dynamic_dma_scratch_size=2048)
    MD = _md(mm)
    D = KO * 128
    QB = 512
    NQH = S // QB
    KC = S // 128
    SA = S + A
    nrep = HL // KVL
    KVD = KVL * HD

    xp = nc.declare_dram_parameter("xp", [128, KO, S], MD, isOutput=False)
    wqp = nc.declare_dram_parameter("wqp", [HL, 128, KO, HD], MD, isOutput=False)
    wkp = nc.declare_dram_parameter("wkp", [KVL, 128, KO, HD], MD, isOutput=False)
    wvp = nc.declare_dram_parameter("wvp", [128, KO, KVD], MD, isOutput=False)
    wop = nc.declare_dram_parameter("wop", [128, HL, D], MD, isOutput=False)
    csp = nc.declare_dram_parameter("csp", [128, 2, S], MD, isOutput=False)
    trip = nc.declare_dram_parameter("trip", [128, 128], MD, isOutput=False)
    akp = nc.declare_dram_parameter("akp", [KVL, 128, A], MD, isOutput=False)
    savp = nc.declare_dram_parameter("savp", [A, HL * HD], MD,
                                     isOutput=False)
    outp = nc.declare_dram_parameter("out", [S // 128, 128, D], MD,
                                     isOutput=True)
    outp2 = nc.declare_dram_parameter("out2", [S // 128, 128, D], MD,
                                      isOutput=True)
    DBG = os.environ.get("KDBG", "") == "1"
    if DBG:
        qTd = nc.declare_dram_parameter("qTd", [128, S], MD, isOutput=True)
        kTd = nc.declare_dram_parameter("kTd", [128, S + A], MD, isOutput=True)
        vvd = nc.declare_dram_parameter("vvd", [128, (S // 128) + 1,
                                                KVL * HD], MD, isOutput=True)
        oTd = nc.declare_dram_parameter("oTd", [HL, 128, S], MD, isOutput=True)

    Exp = mybir.ActivationFunctionType.Exp
    Radd = bass_isa.ReduceOp.add

    with tile.TileContext(nc) as tc:
        with tc.tile_pool(name="singles", bufs=1) as singles, \
             tc.tile_pool(name="persist", bufs=1) as persist, \
             tc.tile_pool(name="wpool", bufs=3) as wpool, \
             tc.tile_pool(name="rpool", bufs=1) as rpool, \
             tc.tile_pool(name="spool", bufs=1) as spool:
            nc.gpsimd.load_library(library_config.attn)

            XG = 4
            NX = KO // XG
            xt = [persist.tile([128, XG, S], MD, tag=f"x{i}", name=f"x{i}")
                  for i in range(NX)]
            x_loaded = [False] * KO
            _xq = [nc.sync]

            def xload(c):
                # three issuers for the early chunks: SP alone can only
                # start a DMA every ~650ns, and the V pass consumes one
                # chunk per ~850ns from t~3us
                if not x_loaded[c]:
                    if c in (1, 2):
                        eng = nc.scalar
                    elif c in (3, 6):
                        eng = nc.gpsimd
                    else:
                        eng = nc.sync
                    eng.dma_start(xt[c // XG][:, c % XG, :], xp[:, c, :])
                    x_loaded[c] = True

            def xsl(c):
                return xt[c // XG][:, c % XG, :]

            # first compute needs wv chunk 0 + the first x tokens: tiny
            # leading DMAs, before the (larger) table DMAs, so no queue
            # serializes startup
            WBV = 4
            wvt = [persist.tile([128, WBV, KVD], MD, tag=f"wv{b}",
                                name=f"wv{b}") for b in range(KO // WBV)]
            # x first on SP (Act's engine warms its activation table at
            # t=0, which can gate its DMA generation); wv mini on Act
            nc.sync.dma_start(xt[0][:, 0, 0:256], xp[:, 0, 0:256])
            nc.scalar.dma_start(wvt[0][:, 0, :], wvp[:, 0, :])
            nc.sync.dma_start(xt[0][:, 0, 256:S], xp[:, 0, 256:S])
            x_loaded[0] = True
            nc.sync.dma_start(wvt[0][:, 1:WBV, :], wvp[:, 1:WBV, :])
            # K0's first weight tile: prefetched now, consumed at ~35us
            kwt0 = wpool.tile([128, 8, HD], MD, tag="w", bufs=9)
            nc.sync.dma_start(kwt0, wkp[0, :, 0:8, :])
            # x and wv streams interleaved at the V pass's consumption
            # ratio (4 x chunks per wv tile) so neither starves the PE
            for c in range(1, KO):
                xload(c)
                if c % WBV == WBV - 2 and c // WBV + 1 < KO // WBV:
                    b = c // WBV + 1
                    nc.sync.dma_start(wvt[b],
                                      wvp[:, b * WBV:(b + 1) * WBV, :])
            tri = singles.tile([128, 128], MD)
            nc.sync.dma_start(tri, trip[:])
            zb = singles.tile([128, 1], F32)
            nc.vector.memset(zb, 0.0)
            csd = singles.tile([128, 2, S], MD)
            nc.sync.dma_start(csd[:, 0, :], csp[:, 0, :])
            nc.sync.dma_start(csd[:, 1, :], csp[:, 1, :])
            csA = csd[:, 0, :]
            csB = csd[:, 1, :]
            kT = [persist.tile([128, SA], MD, tag=f"kT{j}", name=f"kT{j}")
                  for j in range(KVL)]
            qT = [persist.tile([128, S], MD, tag=f"qT{h}", name=f"qT{h}")
                  for h in range(HL)]
            vv = persist.tile([128, KC, KVD], MD, tag="vv")
            sav = persist.tile([A, HL * HD], MD, tag="sav")
            nc.sync.dma_start(sav, savp[:])
            for j in range(KVL):
                nc.sync.dma_start(kT[j][:, S:SA], akp[j])
            oT = [persist.tile([128, S], MD, tag=f"oT{h}", name=f"oT{h}")
                  for h in range(HL)]
            woa = persist.tile([128, HL // 2, D], MD, tag="woa")

            # ---------------- V projection: accumulate in psum ------------
            # wv stays resident (adapter-V re-reads it); t-major emission
            # threads the psum->vv copies between banks so the V->K psum
            # handover doesn't wait on a burst of copies at the end
            with tc.tile_pool(name="psv", bufs=1, space="PSUM") as psv:
                vb = [psv.tile([128, KVD], F32, tag=f"vb{t}", name=f"vb{t}")
                      for t in range(KC)]
                for t in range(KC):
                    for c in range(KO):
                        nc.tensor.matmul(
                            vb[t], xsl(c)[:, t * 128:(t + 1) * 128],
                            wvt[c // WBV][:, c % WBV, :],
                            start=(c == 0), stop=(c == KO - 1))
                    # drain copies split across Act and DVE so the psum
                    # scope handover isn't serialized on one engine
                    if t % 2 == 1:
                        nc.scalar.copy(vv[:, t, :], vb[t])
                    else:
                        nc.vector.tensor_scalar_mul(vv[:, t, :], vb[t], 1.0)

            # ---------------- main pipelined scope ------------------------
            with tc.tile_pool(name="mp", bufs=1, space="PSUM") as mp:

                def emit_rope(ps_h, dst, hh):
                    sl = slice(hh * QB, (hh + 1) * QB)
                    rc = rpool.tile([128, QB], MD, tag="rc", bufs=2)
                    nc.scalar.copy(rc, ps_h)
                    rs = rpool.tile([128, QB], MD, tag="rs", bufs=2)
                    nc.sync.dma_start(rs[0:64, :], rc[64:128, :])
                    nc.sync.dma_start(rs[64:128, :], rc[0:64, :])
                    tm1 = rpool.tile([128, QB], MD, tag="tm1", bufs=2)
                    tm2 = rpool.tile([128, QB], MD, tag="tm2", bufs=2)
                    nc.vector.tensor_mul(tm1, rc, csA[:, sl])
                    nc.vector.tensor_mul(tm2, rs, csB[:, sl])
                    nc.vector.tensor_sub(dst[0:64, sl], tm1[0:64, :],
                                         tm2[0:64, :])
                    nc.vector.tensor_add(dst[64:128, sl], tm2[64:128, :],
                                         tm1[64:128, :])

                WBQ = 8

                def emit_head_proj(wparam, hidx, dst, first_wt=None):
                    ps_b = [mp.tile([128, QB], F32, tag="proj", bufs=2,
                                    name=f"pj{hidx}_{hh}")
                            for hh in range(NQH)]
                    wts = []
                    for b in range(KO // WBQ):
                        if b == 0 and first_wt is not None:
                            wts.append(first_wt)
                        else:
                            wt = wpool.tile([128, WBQ, HD], MD, tag="w",
                                            bufs=9)
                            nc.sync.dma_start(
                                wt, wparam[hidx, :, b * WBQ:(b + 1) * WBQ, :])
                            wts.append(wt)
                    for hh in range(NQH):
                        sl = slice(hh * QB, (hh + 1) * QB)
                        for c in range(KO):
                            xload(c)
                            st, sp = (c == 0), (c == KO - 1)
                            nc.tensor.matmul(ps_b[hh],
                                             wts[c // WBQ][:, c % WBQ, :],
                                             xsl(c)[:, sl],
                                             start=st, stop=sp)
                        emit_rope(ps_b[hh], dst, hh)

                def emit_attn_head(h, qb):
                    """Stage A: scores, exps, masking, denominator
                    accumulation, all-reduces, token PVs.  Returns a closure
                    (stage C) that finishes normalization one step later."""
                    j = h // nrep
                    qs, qe = qb * QB, (qb + 1) * QB
                    chunks = []
                    for kc in range(KC):
                        if kc * 128 >= qe:
                            break
                        q0 = max(qs, kc * 128)
                        chunks.append((kc, q0, qe - q0))
                    pairs = [chunks[i:i + 2]
                             for i in range(0, len(chunks), 2)]
                    entries = []
                    for pair in pairs:
                        scp = mp.tile([128, 2 * QB], F32, tag="scp",
                                      bufs=2)
                        pt = spool.tile([128, 2 * QB], MD, tag="pt",
                                        bufs=4)
                        for si, (kc, q0, N) in enumerate(pair):
                            nc.tensor.matmul(
                                scp[:, si * QB:si * QB + N],
                                kT[j][:, kc * 128:(kc + 1) * 128],
                                qT[h][:, q0:qe], start=True, stop=True)
                        NA = pair[0][2]
                        if len(pair) == 2 and NA == QB:
                            NB = pair[1][2]
                            nc.scalar.activation(pt[:, 0:QB + NB],
                                                 scp[:, 0:QB + NB], Exp,
                                                 bias=zb)
                        else:
                            nc.scalar.activation(pt[:, 0:NA],
                                                 scp[:, 0:NA], Exp,
                                                 bias=zb)
                            if len(pair) == 2:
                                NB = pair[1][2]
                                nc.scalar.activation(
                                    pt[:, QB:QB + NB],
                                    scp[:, QB:QB + NB], Exp, bias=zb)
                        for si, (kc, q0, N) in enumerate(pair):
                            if kc * 128 >= qs:
                                sd = si * QB
                                nc.vector.tensor_mul(
                                    pt[:, sd:sd + 128],
                                    pt[:, sd:sd + 128], tri)
                            entries.append((pt, si * QB, kc, q0, N))
                    # denominator: DVE-accumulate chunk probs, then
                    # partition-sum-broadcast on gpsimd
                    pts = spool.tile([128, QB], MD, tag="ptsum", bufs=2)
                    (p0, s0, _, _, _) = entries[0]
                    (p1, s1, _, _, N1) = entries[1]
                    c1 = QB - N1
                    nc.vector.tensor_add(pts[:, c1:QB],
                                         p0[:, s0 + c1:s0 + QB],
                                         p1[:, s1:s1 + N1])
                    if c1:
                        nc.vector.tensor_scalar_mul(
                            pts[:, 0:c1], p0[:, s0:s0 + c1], 1.0)
                    for (pe_, se_, _, _, Ne_) in entries[2:]:
                        ce = QB - Ne_
                        nc.vector.tensor_add(pts[:, ce:QB],
                                             pts[:, ce:QB],
                                             pe_[:, se_:se_ + Ne_])
                    dtb = spool.tile([128, QB], MD, tag="dtb", bufs=3)
                    nc.gpsimd.partition_all_reduce(dtb, pts, channels=128,
                                                   reduce_op=Radd)
                    ot = mp.tile([128, QB], F32, tag="ot", bufs=2)
                    for ki, (pt, sl_, kc, q0, N) in enumerate(entries):
                        nc.tensor.matmul(
                            ot[:, q0 - qs:QB],
                            vv[:, kc, j * HD:(j + 1) * HD],
                            pt[:, sl_:sl_ + N],
                            start=(ki == 0), stop=False)
                    # adapter block: separately-normalized, gate folded
                    # into sav; accumulates into the same psum bank
                    sca = mp.tile([A, QB], F32, tag="scp", bufs=2)
                    nc.tensor.matmul(sca, kT[j][:, S:SA],
                                     qT[h][:, qs:qe],
                                     start=True, stop=True)
                    pa = spool.tile([A, QB], MD, tag="pa", bufs=3)
                    nc.scalar.activation(pa, sca, Exp, bias=zb[0:A, :])
                    dab = spool.tile([A, QB], MD, tag="dab", bufs=3)
                    nc.gpsimd.partition_all_reduce(dab, pa, channels=A,
                                                   reduce_op=Radd)

                    def finish():
                        with nc.allow_low_precision(
                                reason="fp16 softmax scales, same rounding "
                                       "as every other fp16 operand"):
                            rca = spool.tile([A, QB], MD, tag="rca", bufs=2)
                            nc.vector.reciprocal(rca, dab)
                        # scale by dt/da: the shared psum gets divided by dt
                        # at the end, leaving the adapter block over da only
                        rca2 = spool.tile([A, QB], MD, tag="rca2", bufs=2)
                        nc.vector.tensor_mul(rca2, rca, dtb[0:A, :])
                        pan = spool.tile([A, QB], MD, tag="pan", bufs=2)
                        nc.vector.tensor_mul(pan, pa, rca2)
                        rct = spool.tile([128, QB], MD, tag="rct", bufs=2)
                        with nc.allow_low_precision(
                                reason="fp16 softmax scales, same rounding "
                                       "as every other fp16 operand"):
                            nc.vector.reciprocal(rct, dtb)
                        nc.tensor.matmul(ot,
                                         sav[0:A, h * HD:(h + 1) * HD],
                                         pan, start=False, stop=True)
                        nc.vector.tensor_mul(oT[h][:, qs:qe], ot, rct)

                    return finish

                # software pipeline: stage A of step k runs before the
                # normalization tail (stage C) of step k-1, so the next
                # block's mask/exp work leads the in-order DVE queue
                pending = [None]

                def attn_step(h, qb):
                    fin = emit_attn_head(h, qb)
                    if pending[0] is not None:
                        pending[0]()
                    pending[0] = fin

                emit_head_proj(wkp, 0, kT[0], first_wt=kwt0)
                emit_head_proj(wkp, 1, kT[1])
                # wo weights for heads 0..HL/2-1 (needed last, but queued
                # here so the stream never contends with projection weights)
                for hh in range(HL // 2):
                    for q4 in range(4):
                        nc.sync.dma_start(
                            woa[:, hh, q4 * (D // 4):(q4 + 1) * (D // 4)],
                            wop[:, hh, q4 * (D // 4):(q4 + 1) * (D // 4)])
                emit_head_proj(wqp, 0, qT[0])
                for h in range(1, HL):
                    emit_head_proj(wqp, h, qT[h])
                    attn_step(h - 1, 0)
                    attn_step(h - 1, 1)
                # wo weights for heads HL/2..HL-1 reuse dead x slots
                wob = [persist.tile([128, 1, D], MD, tag=f"x{NX - 4 + i}",
                                    name=f"wob{i}")
                       for i in range(HL // 2)]
                for i in range(HL // 2):
                    for q4 in range(4):
                        nc.sync.dma_start(
                            wob[i][:, 0, q4 * (D // 4):(q4 + 1) * (D // 4)],
                            wop[:, HL // 2 + i,
                                q4 * (D // 4):(q4 + 1) * (D // 4)])
                # last head: qb1 first and eagerly flushed, so the final
                # normalization tails overlap the output projection
                attn_step(HL - 1, 1)
                pending[0]()
                fin_last = emit_attn_head(HL - 1, 0)
                fin_last()

                # ---------------- output projection -----------------------
                # two half-head passes writing separate partial outputs
                # (host sums them): pass A only needs heads 0..HL/2-1, so
                # its matmuls keep the PE fed while the last heads'
                # softmax chains drain on DVE/Act/Pool
                def wsl(hh, n):
                    if hh < HL // 2:
                        return woa[:, hh, n * QB:(n + 1) * QB]
                    return wob[hh - HL // 2][:, 0, n * QB:(n + 1) * QB]

                if DBG:
                    nc.sync.dma_start(qTd[:], qT[0])
                    nc.sync.dma_start(kTd[:], kT[0])
                    nc.sync.dma_start(vvd[:], vv)
                    for h in range(HL):
                        nc.sync.dma_start(oTd[h], oT[h])

                NWO = D // QB
                NM = S // 128
                HH2 = HL // 2
                for half in range(2):
                    od = outp if half == 0 else outp2
                    # second pass: qb1-dependent rows first (they unblock
                    # as soon as the last head's qb1 tail finishes)
                    ms = (list(range(NM)) if half == 0 else
                          list(range(NM // 2, NM)) + list(range(NM // 2)))
                    for m in ms:
                        ob = None
                        for n in range(NWO):
                            pso = mp.tile([128, QB], F32, tag="proj",
                                          bufs=2)
                            for hi in range(HH2):
                                hh = half * HH2 + hi
                                nc.tensor.matmul(
                                    pso, oT[hh][:, m * 128:(m + 1) * 128],
                                    wsl(hh, n),
                                    start=(hi == 0), stop=(hi == HH2 - 1))
                            if ob is None:
                                ob = spool.tile([128, 2, QB], MD, tag="ob",
                                                bufs=5)
                            last = (half == 1 and n == NWO - 1
                                    and m == ms[-1])
                            nc.scalar.copy(ob[:, n % 2, :], pso)
                            if n % 2 == 1 and not last:
                                # one DMA per two tiles: halves the SP
                                # sequencer's per-DMA issue load
                                nc.sync.dma_start(
                                    od[m, :, (n - 1) * QB:(n + 1) * QB],
                                    ob[:, 0:2, :])
                                ob = None
                            elif last:
                                nc.sync.dma_start(
                                    od[m, :, (n - 1) * QB:n * QB],
                                    ob[:, 0, :])
                                nc.sync.dma_start(
                                    od[m, :, n * QB:(n + 1) * QB],
                                    ob[:, 1, :])

    nc.compile()
    nc.finalize()
    return nc


# --------------------------------------------------------------------------
# device program (legacy general-mask path)
# --------------------------------------------------------------------------

def build_program(KO, S, HL, KVL, causal, mm):
    """One NeuronCore's program.

    KO: D // 128 contraction chunks.  S: sequence length.  HL: q heads on
    this core.  KVL: kv heads on this core.  causal: hardwire causal
    masking (tri mask on diagonal chunks + chunk skipping); otherwise an
    additive mask [S, S] is an input.  mm: matmul operand dtype mode.
    """
    nc = bacc.Bacc(None, target_bir_lowering=False,
                   dynamic_dma_scratch_size=2048)
    MD = _md(mm)
    D = KO * 128
    QB = min(512, S)       # q column block (psum bank + fp32 moving max)
    NQH = S // QB
    KC = S // 128          # token key chunks
    SA = S + A
    nrep = HL // KVL

    xp = nc.declare_dram_parameter("xp", [128, KO, S], MD, isOutput=False)
    wqp = nc.declare_dram_parameter("wqp", [HL, 128, KO, HD], MD, isOutput=False)
    wkp = nc.declare_dram_parameter("wkp", [KVL, 128, KO, HD], MD, isOutput=False)
    wvp = nc.declare_dram_parameter("wvp", [128, KO, KVL * HD], MD, isOutput=False)
    wop = nc.declare_dram_parameter("wop", [128, HL, D], MD, isOutput=False)
    adp = nc.declare_dram_parameter("adp", [128, KO, A], MD, isOutput=False)
    csp = nc.declare_dram_parameter("csp", [128, 2, S], F32, isOutput=False)
    trip = nc.declare_dram_parameter("trip", [128, 128], MD, isOutput=False)
    gcp = nc.declare_dram_parameter("gcp", [1, HL * 128], MD, isOutput=False)
    if not causal:
        mtp = nc.declare_dram_parameter("mtp", [128, KC, S], F32, isOutput=False)
    outp = nc.declare_dram_parameter("out", [S // 128, 128, D], F32, isOutput=True)
    HSPLIT = 1
    if HSPLIT == 2:
        outp2 = nc.declare_dram_parameter("out2", [S // 128, 128, D], F32,
                                          isOutput=True)

    Exp = mybir.ActivationFunctionType.Exp

    with tile.TileContext(nc) as tc:
        with tc.tile_pool(name="singles", bufs=1) as singles, \
             tc.tile_pool(name="persist", bufs=1) as persist:
            tri = singles.tile([128, 128], MD)
            nc.sync.dma_start(tri, trip[:])

            # all-ones vectors: row 0 / column 127 of the tri mask
            ones_row = tri[0:1, :]
            ones_col = tri[:, 127:128]
            zb = singles.tile([128, 1], F32)
            nc.vector.memset(zb, 0.0)

            # resident x^T in XG-chunk tiles, DMA'd just-in-time from the
            # V-projection loop so the first matmuls start early
            XG = min(4, KO)
            NX = KO // XG
            xt = [persist.tile([128, XG, S], MD, tag=f"x{i}", name=f"x{i}")
                  for i in range(NX)]
            xt_loaded = [False] * NX

            def xload(i):
                if not xt_loaded[i]:
                    h = XG // 2 or 1
                    nc.sync.dma_start(xt[i][:, 0:h, :],
                                      xp[:, i * XG:i * XG + h, :])
                    if h < XG:
                        nc.sync.dma_start(xt[i][:, h:XG, :],
                                          xp[:, i * XG + h:(i + 1) * XG, :])
                    xt_loaded[i] = True

            def xsl(c):
                return xt[c // XG][:, c % XG, :]

            kT = [persist.tile([128, SA], MD, tag=f"kT{j}", name=f"kT{j}")
                  for j in range(KVL)]
            vv = persist.tile([128, KC + 1, KVL * HD], MD, tag="vv")
            qT = [persist.tile([128, S], MD, tag=f"qT{h}", name=f"qT{h}")
                  for h in range(HL)]

            # ---------------- phase 1: projections -----------------------
            with tc.tile_pool(name="wpool", bufs=3) as wpool, \
                 tc.tile_pool(name="rpool", bufs=2) as rpool, \
                 tc.tile_pool(name="cpool", bufs=1) as cpool:
                # csA: cos^T duplicated on both partition halves; csB: sin^T
                csd = cpool.tile([128, 2, S], F32)
                nc.sync.dma_start(csd, csp[:])
                csA = csd[:, 0, :]
                csB = csd[:, 1, :]
                adT = cpool.tile([128, KO, A], MD)
                nc.sync.dma_start(adT, adp[:])

                def emit_rope(ps_h, dst, hh):
                    # psum rows 0:64 = x0 (even pair elems), 64:128 = x1.
                    # dst[0:64] = x0*cos - x1*sin ; dst[64:128] = x0*sin + x1*cos
                    sl = slice(hh * QB, (hh + 1) * QB)
                    rc = rpool.tile([128, QB], F32, tag="rc", bufs=2)
                    nc.scalar.copy(rc, ps_h)        # frees the psum slot fast
                    rs = rpool.tile([128, QB], F32, tag="rs", bufs=2)
                    nc.sync.dma_start(rs[0:64, :], rc[64:128, :])
                    nc.sync.dma_start(rs[64:128, :], rc[0:64, :])
                    # tm1 = [x0*cos ; x1*cos], tm2 = [x1*sin ; x0*sin]
                    tm1 = rpool.tile([128, QB], F32, tag="tm1", bufs=1)
                    tm2 = rpool.tile([128, QB], F32, tag="tm2", bufs=1)
                    nc.vector.tensor_mul(tm1, rc, csA[:, sl])
                    nc.vector.tensor_mul(tm2, rs, csB[:, sl])
                    nc.vector.tensor_sub(dst[0:64, sl], tm1[0:64, :], tm2[0:64, :])
                    nc.vector.tensor_add(dst[64:128, sl], tm2[64:128, :],
                                         tm1[64:128, :])

                ps1cm = tc.tile_pool(name="ps1", bufs=1, space="PSUM")
                ps1 = ps1cm.__enter__()

                # ---- V projection (token-major): stream wv once; accumulate
                # the cross-block partials in an f32 SBUF tile so psum needs
                # only 2 banks.  Adapter V accumulates in its own bank.
                WBV = min(4, KO)
                NVB = KO // WBV
                vacc = cpool.tile([128, KC, KVL * HD], F32)
                pav = ps1.tile([A, KVL * HD], F32, tag="av")

                def emit_vblock(b):
                    wt = wpool.tile([128, WBV, KVL * HD], MD, tag="w")
                    nc.sync.dma_start(wt, wvp[:, b * WBV:(b + 1) * WBV, :])
                    for i in range(b * WBV // XG,
                                   (b * WBV + WBV - 1) // XG + 1):
                        xload(i)
                    for t in range(KC):
                        psv = ps1.tile([128, KVL * HD], F32, tag="vproj",
                                       bufs=2)
                        for ci in range(WBV):
                            c = b * WBV + ci
                            nc.tensor.matmul(
                                psv[:, :], xsl(c)[:, t * 128:(t + 1) * 128],
                                wt[:, ci, :],
                                start=(ci == 0), stop=(ci == WBV - 1))
                        if b == 0 and NVB > 1:
                            nc.scalar.copy(vacc[:, t, :], psv[:, :])
                        elif b < NVB - 1:
                            nc.vector.tensor_add(vacc[:, t, :], vacc[:, t, :],
                                                 psv[:, :])
                        elif NVB > 1:
                            nc.vector.tensor_add(vv[:, t, :], vacc[:, t, :],
                                                 psv[:, :])
                        else:
                            nc.scalar.copy(vv[:, t, :], psv[:, :])
                    for ci in range(WBV):
                        c = b * WBV + ci
                        nc.tensor.matmul(pav[:, :], adT[:, c, :], wt[:, ci, :],
                                         start=(c == 0), stop=(c == KO - 1))
                    if b == NVB - 1:
                        nc.scalar.copy(vv[0:A, KC, :], pav[:, :])

                WBQ = min(8, KO)

                def emit_khead(j):
                    psk = [ps1.tile([128, QB], F32, tag="proj", bufs=4,
                                    name=f"psk{hh}") for hh in range(NQH)]
                    pak = ps1.tile([128, A], F32, tag="ak")
                    for b in range(KO // WBQ):
                        wt = wpool.tile([128, WBQ, HD], MD, tag="w")
                        nc.sync.dma_start(wt, wkp[j, :, b * WBQ:(b + 1) * WBQ, :])
                        for i in range(b * WBQ // XG,
                                       (b * WBQ + WBQ - 1) // XG + 1):
                            xload(i)
                        for ci in range(WBQ):
                            c = b * WBQ + ci
                            st, sp = (c == 0), (c == KO - 1)
                            for hh in range(NQH):
                                sl = slice(hh * QB, (hh + 1) * QB)
                                nc.tensor.matmul(
                                    psk[hh][:, :], wt[:, ci, :], xsl(c)[:, sl],
                                    start=st, stop=sp)
                            nc.tensor.matmul(
                                pak[:, :], wt[:, ci, :], adT[:, c, :],
                                start=st, stop=sp)
                    for hh in range(NQH):
                        emit_rope(psk[hh], kT[j], hh)
                    nc.scalar.copy(kT[j][:, S:SA], pak[:, 0:A])

                def emit_qhead(h):
                    psq = [ps1.tile([128, QB], F32, tag="proj", bufs=4,
                                    name=f"psq{hh}") for hh in range(NQH)]
                    for b in range(KO // WBQ):
                        wt = wpool.tile([128, WBQ, HD], MD, tag="w")
                        nc.sync.dma_start(wt, wqp[h, :, b * WBQ:(b + 1) * WBQ, :])
                        for i in range(b * WBQ // XG,
                                       (b * WBQ + WBQ - 1) // XG + 1):
                            xload(i)
                        for ci in range(WBQ):
                            c = b * WBQ + ci
                            st, sp = (c == 0), (c == KO - 1)
                            for hh in range(NQH):
                                sl = slice(hh * QB, (hh + 1) * QB)
                                nc.tensor.matmul(
                                    psq[hh][:, :], wt[:, ci, :], xsl(c)[:, sl],
                                    start=st, stop=sp)
                    for hh in range(NQH):
                        emit_rope(psq[hh], qT[h], hh)

                # Interleave V blocks between K/Q head projections so the
                # DMA-heavy V stream overlaps compute-heavy head projections.
                kq = [("k", j) for j in range(KVL)] + \
                     [("q", h) for h in range(HL)]
                vb = list(range(NVB))
                seq = []
                while vb or kq:
                    if vb:
                        seq.append(("v", vb.pop(0)))
                    if kq:
                        seq.append(kq.pop(0))
                for kind, idx in seq:
                    if kind == "v":
                        emit_vblock(idx)
                    elif kind == "k":
                        emit_khead(idx)
                    else:
                        emit_qhead(idx)
                ps1cm.__exit__(None, None, None)

            # ---------------- phase 2: attention --------------------------
            # oT / wo-weights / general-mask reuse the dead x-tile slots
            HG = min(4, HL)
            oTt = [persist.tile([128, HG, S], MD,
                                tag=(f"x{i}" if i < NX else f"oT{i}"),
                                name=f"oTall{i}")
                   for i in range((HL + HG - 1) // HG)]

            def oT(h):
                return oTt[h // HG][:, h % HG, :]

            mt = None
            if not causal:
                mtt = [persist.tile([128, KC // 2, S], F32,
                                    tag=(f"x{4 + i}" if NX > 5 else f"mt{i}"),
                                    name=f"mt{i}")
                       for i in range(2)]
                nc.sync.dma_start(mtt[0], mtp[:, 0:KC // 2, :])
                nc.sync.dma_start(mtt[1], mtp[:, KC // 2:KC, :])

                def mtsl(kc):
                    return mtt[kc // (KC // 2)][:, kc % (KC // 2), :]
            with tc.tile_pool(name="spool", bufs=3) as spool, \
                 tc.tile_pool(name="ps2", bufs=1, space="PSUM") as ps2:
                gc = spool.tile([1, HL * 128], MD, tag="gc", bufs=1)
                nc.sync.dma_start(gc, gcp[:])
                for h in range(HL):
                    j = h // nrep
                    for qh in range(NQH):
                        qs, qe = qh * QB, (qh + 1) * QB
                        if causal:
                            kcs = [kc for kc in range(KC) if kc * 128 < qe]
                        else:
                            kcs = list(range(KC))
                        ot_ps = ps2.tile([128, QB], F32, tag="ot", bufs=2)
                        oa_ps = ps2.tile([128, QB], F32, tag="oa", bufs=1)
                        dt_ps = ps2.tile([1, QB], F32, tag="dt", bufs=1)
                        da_ps = ps2.tile([1, QB], F32, tag="da", bufs=1)
                        for ki, kc in enumerate(kcs):
                            q0 = max(qs, kc * 128) if causal else qs
                            N = qe - q0
                            st, sp = (ki == 0), (ki == len(kcs) - 1)
                            scp = ps2.tile([128, QB], F32, tag="scp", bufs=2)
                            nc.tensor.matmul(
                                scp[:, 0:N],
                                kT[j][:, kc * 128:(kc + 1) * 128],
                                qT[h][:, q0:qe], start=True, stop=True)
                            pt = spool.tile([128, QB], MD, tag="pt", bufs=4)
                            if causal:
                                nc.scalar.activation(pt[:, 0:N], scp[:, 0:N],
                                                     Exp, bias=zb)
                                if kc * 128 >= qs:  # diagonal chunk
                                    nc.vector.tensor_mul(
                                        pt[:, 0:128], pt[:, 0:128], tri)
                            else:
                                sadd = spool.tile([128, QB], F32, tag="sadd",
                                                  bufs=2)
                                nc.vector.tensor_add(
                                    sadd[:, 0:N], scp[:, 0:N],
                                    mtsl(kc)[:, q0:qe])
                                nc.scalar.activation(pt[:, 0:N], sadd[:, 0:N],
                                                     Exp, bias=zb)
                            nc.tensor.matmul(
                                ot_ps[:, q0 - qs:QB],
                                vv[:, kc, j * HD:(j + 1) * HD],
                                pt[:, 0:N], start=st, stop=sp)
                            nc.tensor.matmul(
                                dt_ps[0:1, q0 - qs:QB], ones_col[:, 0:1],
                                pt[:, 0:N], start=st, stop=sp)
                        # adapter block
                        sca = ps2.tile([128, QB], F32, tag="scp", bufs=2)
                        nc.tensor.matmul(sca[0:A, :], kT[j][:, S:SA],
                                         qT[h][:, qs:qe], start=True, stop=True)
                        pa = spool.tile([128, QB], MD, tag="pt", bufs=4)
                        nc.scalar.activation(pa[0:A, :], sca[0:A, :], Exp,
                                             bias=zb[0:A, :])
                        nc.tensor.matmul(oa_ps[:, :],
                                         vv[0:A, KC, j * HD:(j + 1) * HD],
                                         pa[0:A, :], start=True, stop=True)
                        nc.tensor.matmul(da_ps[0:1, :], ones_col[0:A, 0:1],
                                         pa[0:A, :], start=True, stop=True)
                        # normalization factors (per-q scalars), f32r direct
                        rt = spool.tile([1, QB], MD, tag="rt", bufs=1)
                        ra = spool.tile([1, QB], MD, tag="ra", bufs=1)
                        with nc.allow_low_precision(
                                reason="f32r softmax scales, rounded like "
                                       "every other matmul operand"):
                            nc.vector.reciprocal(rt, dt_ps[0:1, :])
                            nc.vector.reciprocal(ra, da_ps[0:1, :])
                        # broadcast across partitions via rank-1 matmul;
                        # tanh(gate_h) is folded into the adapter lhsT (gc)
                        rp1 = ps2.tile([128, QB], F32, tag="rp", bufs=1)
                        nc.tensor.matmul(rp1, ones_row[0:1, :], rt[0:1, :],
                                         start=True, stop=True)
                        rtb = spool.tile([128, QB], F32, tag="rtb", bufs=1)
                        nc.scalar.copy(rtb, rp1)
                        rp2 = ps2.tile([128, QB], F32, tag="rp", bufs=1)
                        nc.tensor.matmul(rp2, gc[0:1, h * 128:(h + 1) * 128],
                                         ra[0:1, :], start=True, stop=True)
                        rab = spool.tile([128, QB], F32, tag="rab", bufs=1)
                        nc.scalar.copy(rab, rp2)
                        # oT = ot/denom_t + tanh(g)*oa/denom_a  (write-once)
                        tq1 = spool.tile([128, QB], F32, tag="tq1", bufs=1)
                        nc.vector.tensor_mul(tq1, ot_ps[:, :], rtb)
                        tq2 = spool.tile([128, QB], F32, tag="tq2", bufs=1)
                        nc.vector.tensor_mul(tq2, oa_ps[:, :], rab)
                        nc.vector.tensor_add(oT(h)[:, qs:qe], tq1, tq2)

            # ---------------- phase 3: output projection ------------------
            # Split into two half-head passes writing separate partial
            # outputs (host sums them): the first pass only needs heads
            # 0..HL/2-1 and overlaps the tail of the attention phase.
            with tc.tile_pool(name="wopool", bufs=2) as wopool, \
                 tc.tile_pool(name="obpool", bufs=3) as obpool, \
                 tc.tile_pool(name="ps3", bufs=4, space="PSUM") as ps3:
                NB = D // 512
                HH = HL // HSPLIT
                # deep prefetch of wo weight tiles into freed x slots
                slots = [i for i in range(2, NX)
                         if causal or i not in (4, 5)] or [None]
                for half in range(HSPLIT):
                    od = outp if half == 0 else outp2
                    h0 = half * HH
                    for n in range(NB):
                        wi = slots[(half * NB + n) % len(slots)]
                        if wi is not None and NX > wi:
                            won = persist.tile([128, HH, 512], MD,
                                               tag=f"x{wi}",
                                               name=f"won{half}_{n}")
                        else:
                            won = wopool.tile([128, HH, 512], MD, tag="won")
                        for wpi in range(0, HH, 2):
                            nc.sync.dma_start(
                                won[:, wpi:wpi + 2, :],
                                wop[:, h0 + wpi:h0 + wpi + 2,
                                    n * 512:(n + 1) * 512])
                        for m in range(S // 128):
                            pso = ps3.tile([128, 512], F32, tag="wo", bufs=6)
                            for hh2 in range(HH):
                                nc.tensor.matmul(
                                    pso,
                                    oT(h0 + hh2)[:, m * 128:(m + 1) * 128],
                                    won[:, hh2, :],
                                    start=(hh2 == 0), stop=(hh2 == HH - 1))
                            ob = obpool.tile([128, 512], F32, tag="ob")
                            nc.scalar.copy(ob, pso)
                            nc.sync.dma_start(
                                od[m, :, n * 512:(n + 1) * 512], ob)

    nc.compile()
    nc.finalize()
    return nc


def get_program(KO, S, HL, KVL, causal, mm):
    key = (KO, S, HL, KVL, causal, mm)
    if key not in _PROG_CACHE:
        if causal:
            _PROG_CACHE[key] = build_program_v2(KO, S, HL, KVL, mm)
        else:
            _PROG_CACHE[key] = build_program(KO, S, HL, KVL, causal, mm)
    return _PROG_CACHE[key]


# --------------------------------------------------------------------------
# host-side sharding / layout prep
# --------------------------------------------------------------------------

_EVEN_FIRST = np.concatenate([np.arange(0, HD, 2), np.arange(1, HD, 2)])


def is_causal_mask(mask):
    S = mask.shape[-1]
    m = np.asarray(mask).reshape(S, S)
    iu = np.triu_indices(S, 1)
    il = np.tril_indices(S)
    return bool(np.all(m[il] == 0.0) and np.all(m[iu] <= -1e8))


def _np_md(mm):
    if mm == "bf16":
        import ml_dtypes
        return ml_dtypes.bfloat16
    if mm == "fp16":
        return np.float16
    return np.float32


def prep_core_inputs_v2(core, G, x, wq, wk, wv, wo, adapter, gate,
                        freqs_cos, freqs_sin, mm=None):
    """v2 layout: fp16 tensors, tanh(gate) table instead of gcp, single
    fp16 partial output per core."""
    mm = MM_MODE if mm is None else mm
    B, S, D = x.shape
    H = gate.shape[1]
    hd = wq.shape[1] // H
    KV = wk.shape[1] // hd
    KO = D // 128
    HL, KVL = H // G, KV // G
    b, g = core // G, core % G
    hsl = slice(g * HL, (g + 1) * HL)
    ksl = slice(g * KVL, (g + 1) * KVL)
    idx = _EVEN_FIRST
    f32 = np.float32
    md = _np_md(mm)

    def c(a, dt=None):
        return np.ascontiguousarray(a, dtype=dt if dt is not None else md)

    xp = c(x[b].T.reshape(KO, 128, S).transpose(1, 0, 2))
    wq4 = wq.reshape(D, H, hd)[:, hsl][:, :, idx] * np.float32(1.0 / np.sqrt(hd))
    wqp = c(wq4.reshape(KO, 128, HL, hd).transpose(2, 1, 0, 3))
    wk4 = wk.reshape(D, KV, hd)[:, ksl][:, :, idx]
    wkp = c(wk4.reshape(KO, 128, KVL, hd).transpose(2, 1, 0, 3))
    wv4 = wv.reshape(D, KV, hd)[:, ksl]
    wvp = c(wv4.reshape(KO, 128, KVL * hd).transpose(1, 0, 2))
    wos = wo[g * HL * hd:(g + 1) * HL * hd]
    wop = c(wos.reshape(HL, hd, D).transpose(1, 0, 2))
    ad = np.asarray(adapter[0], dtype=f32)
    ak = np.einsum("ad,dje->jea", ad, wk4.astype(f32))       # [KVL, hd, A]
    av = np.einsum("ad,dje->aje", ad, wv4.astype(f32))       # [A, KVL, hd]
    gth = np.tanh(np.asarray(gate[0, hsl, 0, 0], dtype=np.float64))
    nrep = (wq.shape[1] // hd) // KV * 1  # q heads per kv head (global)
    nrep = HL // KVL
    savp = np.empty((A, HL * hd), f32)
    for h in range(HL):
        savp[:, h * hd:(h + 1) * hd] = av[:, h // nrep, :] * gth[h]
    ct = np.asarray(freqs_cos, dtype=f32).T
    st = np.asarray(freqs_sin, dtype=f32).T
    csp = np.empty((128, 2, S), f32)
    csp[0:64, 0] = ct
    csp[64:128, 0] = ct
    csp[0:64, 1] = st
    csp[64:128, 1] = st
    tri = c(np.triu(np.ones((128, 128), dtype=f32)))
    return {"xp": xp, "wqp": wqp, "wkp": wkp, "wvp": wvp, "wop": wop,
            "csp": c(csp), "trip": tri, "akp": c(ak), "savp": c(savp)}


def prep_core_inputs(core, G, x, wq, wk, wv, wo, adapter, gate,
                     freqs_cos, freqs_sin, mask, causal, mm=None):
    """Build the input dict for one core = (batch b, head-group g)."""
    mm = MM_MODE if mm is None else mm
    B, S, D = x.shape
    H = gate.shape[1]
    hd = wq.shape[1] // H
    KV = wk.shape[1] // hd
    KO = D // 128
    KC = S // 128
    HL, KVL = H // G, KV // G
    b, g = core // G, core % G
    hsl = slice(g * HL, (g + 1) * HL)
    ksl = slice(g * KVL, (g + 1) * KVL)
    idx = _EVEN_FIRST
    f32 = np.float32
    md = _np_md(mm)

    def c(a, dt=None):
        return np.ascontiguousarray(a, dtype=dt if dt is not None else md)

    xp = c(x[b].T.reshape(KO, 128, S).transpose(1, 0, 2))
    wq4 = wq.reshape(D, H, hd)[:, hsl][:, :, idx] * np.float32(1.0 / np.sqrt(hd))
    wqp = c(wq4.reshape(KO, 128, HL, hd).transpose(2, 1, 0, 3))
    wk4 = wk.reshape(D, KV, hd)[:, ksl][:, :, idx]
    wkp = c(wk4.reshape(KO, 128, KVL, hd).transpose(2, 1, 0, 3))
    wv4 = wv.reshape(D, KV, hd)[:, ksl]
    wvp = c(wv4.reshape(KO, 128, KVL * hd).transpose(1, 0, 2))
    wos = wo[g * HL * hd:(g + 1) * HL * hd]
    wop = c(wos.reshape(HL, hd, D).transpose(1, 0, 2))
    adp = c(adapter[0].T.reshape(KO, 128, A).transpose(1, 0, 2))
    # cos^T / sin^T, each duplicated across both partition halves
    ct = np.asarray(freqs_cos, dtype=f32).T      # [64, S]
    st = np.asarray(freqs_sin, dtype=f32).T
    csp = np.empty((128, 2, S), f32)
    csp[0:64, 0] = ct
    csp[64:128, 0] = ct
    csp[0:64, 1] = st
    csp[64:128, 1] = st
    tri = c(np.triu(np.ones((128, 128), dtype=f32)))
    gth = np.tanh(np.asarray(gate[0, hsl, 0, 0], dtype=np.float64)).astype(f32)
    gcp = c(np.repeat(gth, 128).reshape(1, HL * 128))
    inp = {"xp": xp, "wqp": wqp, "wkp": wkp, "wvp": wvp, "wop": wop,
           "adp": adp, "csp": csp, "trip": tri, "gcp": gcp}
    if not causal:
        mt = np.asarray(mask).reshape(S, S).T  # [keys, q]
        inp["mtp"] = c(mt.reshape(KC, 128, S).transpose(1, 0, 2), f32)
    return inp


# --------------------------------------------------------------------------
# entry point
# --------------------------------------------------------------------------

def kernel(x, wq, wk, wv, wo, adapter, gate, freqs_cos, freqs_sin, mask,
           _trace=False):
    x, wq, wk, wv, wo, adapter, gate, freqs_cos, freqs_sin, mask = (
        np.asarray(a) for a in
        (x, wq, wk, wv, wo, adapter, gate, freqs_cos, freqs_sin, mask))
    B, S, D = x.shape
    H = gate.shape[1]
    hd = wq.shape[1] // H
    KV = wk.shape[1] // hd
    G = 8 // B                      # head groups per batch over 8 cores
    HL, KVL = H // G, KV // G
    KO = D // 128

    causal = is_causal_mask(mask)
    nc = get_program(KO, S, HL, KVL, causal, MM_MODE)

    if causal:
        in_maps = [prep_core_inputs_v2(core, G, x, wq, wk, wv, wo, adapter,
                                       gate, freqs_cos, freqs_sin)
                   for core in range(8)]
    else:
        in_maps = [prep_core_inputs(core, G, x, wq, wk, wv, wo, adapter, gate,
                                    freqs_cos, freqs_sin, mask, causal)
                   for core in range(8)]
    res = run_bass_kernel_spmd(nc, in_maps, core_ids=list(range(8)),
                               trace=_trace)
    out = np.zeros((B, S, D), np.float32)
    for core in range(8):
        b = core // G
        r = res.results[core]
        out[b] += r["out"].astype(np.float32).reshape(S, D)
        if "out2" in r:
            out[b] += r["out2"].astype(np.float32).reshape(S, D)
    if _trace:
        kernel._last_result = res
    return out



# revision 82
# speedup vs baseline: 1.0002x; 1.0002x over previous
"""Trainium2 Bass kernel for nn_Attention_50216757625003.

GQA attention layer: B=2, S=1024, D=4096, H=32 q-heads, KV=8 kv-heads,
hd=128, A=10 gated adapter tokens, RoPE, split softmax (adapter block
softmaxed separately and scaled by tanh(gate)), causal mask.

Sharding (8 NeuronCores): outer data-parallel over batch (2) x
tensor-parallel over heads (4 groups of 8 q-heads / 2 kv-heads).
wq/wk/wv are sharded column-wise, wo row-wise; each core computes
partial [S, D] output contributions (two half-head passes) and the host
sums the partials per batch element.

v2 design (causal fast path, ~339us vs the 433us f32r baseline):
  * fp16 operands everywhere: same 1 cycle/row PE rate as bf16 but 8x
    tighter rounding, half the DMA bytes of f32, and it unlocks the
    2x/4x DVE fast modes for packed 2-byte SBUF operands.
  * x is fed transposed ([D, S]); V is projected first, accumulating
    straight into all 8 psum banks (no psum->sbuf->psum round trip),
    then K and Q heads stream through 2 rotating psum banks with the
    two 512-token blocks emitted back-to-back so each bank's drain hides
    under the next block.
  * RoPE: q/k head dims are host-permuted even-first so rotation pairs
    sit on partitions p/p+64; an SBUF->SBUF DMA swaps the halves and
    four fp16 DVE ops apply the duplicated cos/sin tables.
  * scores are built transposed ([keys, q]) in 2-chunk-batched psum
    tiles ([128,1024]) so one Activation exp covers two key chunks.
    Probs stay fp16 and feed the PV matmuls directly.
  * softmax denominators never touch the PE: prob chunks are summed on
    DVE (2x fp16 tensor ops), partition-reduced+broadcast by the gpsimd
    attn-library partition_all_reduce (idle Pool engine), reciprocal'd
    on DVE, and applied as one fused multiply into the output tile.
    The adapter K/V projections (tiny, input-only) are computed on the
    host in f32 -- tanh(gate) pre-folded into the shipped adapter-V --
    and the adapter block rides the same psum accumulation, scaled by
    dt/da so the final 1/dt multiply leaves it normalized by its own
    denominator.
  * attention is software-pipelined: each (head, block) emits scores/
    exp/mask/denominator work, and its normalization tail is emitted
    one step later so the next block's mask ops lead the in-order DVE
    queue; heads interleave with the remaining Q projections.
  * the output projection runs as two half-head passes so its matmuls
    backfill the PE while the last heads' softmax chains drain; output
    DMAs are pair-batched ([128,1024] per DMA) because each dma_start
    costs the issuing sequencer ~650ns+.
  * KMM env still switches "fp16"/"bf16"/"f32r"/"f32" (non-causal masks
    fall back to the legacy general-mask program).
"""

import os
import sys

import numpy as np

for _p in ("/opt/trn_rl_repo",):
    if _p not in sys.path and os.path.isdir(_p):
        sys.path.insert(0, _p)

import concourse.bass as bass
import concourse.bass_isa as bass_isa
import concourse.mybir as mybir
from concourse import bacc
from concourse import library_config
import concourse.tile as tile
from concourse.bass_utils import run_bass_kernel_spmd

HD = 128  # head dim (hardcoded: rope split + tile shapes assume 128)
A = 10    # adapter tokens
F32 = mybir.dt.float32

MM_MODE = os.environ.get("KMM", "fp16")

_PROG_CACHE = {}


def _md(mm):
    return {"f32r": mybir.dt.float32r, "f32": mybir.dt.float32,
            "bf16": mybir.dt.bfloat16, "fp16": mybir.dt.float16}[mm]


# --------------------------------------------------------------------------
# device program (v2, causal-only fast path)
# --------------------------------------------------------------------------

def build_program_v2(KO, S, HL, KVL, mm):
    """Pipelined causal program.

    Engine split per (head, q-block): PE does scores/PV/projection matmuls
    only; Activation does exps (batched over psum pairs); DVE accumulates
    the softmax denominator from prob chunks and applies the reciprocal;
    the gpsimd (Pool) engine broadcasts partition sums via
    partition_all_reduce (attn ucode library), eliminating the baseline's
    denominator/broadcast matmuls and psum->sbuf copies.
    """
    nc = bacc.Bacc(None, target_bir_lowering=False,
                   dynamic_dma_scratch_size=2048)
    MD = _md(mm)
    D = KO * 128
    QB = 512
    NQH = S // QB
    KC = S // 128
    SA = S + A
    nrep = HL // KVL
    KVD = KVL * HD

    xp = nc.declare_dram_parameter("xp", [128, KO, S], MD, isOutput=False)
    wqp = nc.declare_dram_parameter("wqp", [HL, 128, KO, HD], MD, isOutput=False)
    wkp = nc.declare_dram_parameter("wkp", [KVL, 128, KO, HD], MD, isOutput=False)
    wvp = nc.declare_dram_parameter("wvp", [128, KO, KVD], MD, isOutput=False)
    wop = nc.declare_dram_parameter("wop", [128, HL, D], MD, isOutput=False)
    csp = nc.declare_dram_parameter("csp", [128, 2, S], MD, isOutput=False)
    trip = nc.declare_dram_parameter("trip", [128, 128], MD, isOutput=False)
    akp = nc.declare_dram_parameter("akp", [KVL, 128, A], MD, isOutput=False)
    savp = nc.declare_dram_parameter("savp", [A, HL * HD], MD,
                                     isOutput=False)
    outp = nc.declare_dram_parameter("out", [S // 128, 128, D], MD,
                                     isOutput=True)
    outp2 = nc.declare_dram_parameter("out2", [S // 128, 128, D], MD,
                                      isOutput=True)
    DBG = os.environ.get("KDBG", "") == "1"
    if DBG:
        qTd = nc.declare_dram_parameter("qTd", [128, S], MD, isOutput=True)
        kTd = nc.declare_dram_parameter("kTd", [128, S + A], MD, isOutput=True)
        vvd = nc.declare_dram_parameter("vvd", [128, (S // 128) + 1,
                                                KVL * HD], MD, isOutput=True)
        oTd = nc.declare_dram_parameter("oTd", [HL, 128, S], MD, isOutput=True)

    Exp = mybir.ActivationFunctionType.Exp
    Radd = bass_isa.ReduceOp.add

    with tile.TileContext(nc) as tc:
        with tc.tile_pool(name="singles", bufs=1) as singles, \
             tc.tile_pool(name="persist", bufs=1) as persist, \
             tc.tile_pool(name="wpool", bufs=3) as wpool, \
             tc.tile_pool(name="rpool", bufs=1) as rpool, \
             tc.tile_pool(name="spool", bufs=1) as spool:
            nc.gpsimd.load_library(library_config.attn)

            XG = 4
            NX = KO // XG
            xt = [persist.tile([128, XG, S], MD, tag=f"x{i}", name=f"x{i}")
                  for i in range(NX)]
            x_loaded = [False] * KO
            _xq = [nc.sync]

            def xload(c):
                # three issuers for the early chunks: SP alone can only
                # start a DMA every ~650ns, and the V pass consumes one
                # chunk per ~850ns from t~3us
                if not x_loaded[c]:
                    if c in (1, 2):
                        eng = nc.scalar
                    elif c in (3, 6):
                        eng = nc.gpsimd
                    else:
                        eng = nc.sync
                    eng.dma_start(xt[c // XG][:, c % XG, :], xp[:, c, :])
                    x_loaded[c] = True

            def xsl(c):
                return xt[c // XG][:, c % XG, :]

            # first compute needs wv chunk 0 + the first x tokens: tiny
            # leading DMAs, before the (larger) table DMAs, so no queue
            # serializes startup
            WBV = 4
            wvt = [persist.tile([128, WBV, KVD], MD, tag=f"wv{b}",
                                name=f"wv{b}") for b in range(KO // WBV)]
            # x first on SP (Act's engine warms its activation table at
            # t=0, which can gate its DMA generation); wv mini on Act
            nc.sync.dma_start(xt[0][:, 0, 0:256], xp[:, 0, 0:256])
            nc.scalar.dma_start(wvt[0][:, 0, :], wvp[:, 0, :])
            nc.sync.dma_start(xt[0][:, 0, 256:S], xp[:, 0, 256:S])
            x_loaded[0] = True
            nc.sync.dma_start(wvt[0][:, 1:WBV, :], wvp[:, 1:WBV, :])
            # K0's first weight tile: prefetched now, consumed at ~35us
            kwt0 = wpool.tile([128, 8, HD], MD, tag="w", bufs=8)
            nc.sync.dma_start(kwt0, wkp[0, :, 0:8, :])
            # x and wv streams interleaved at the V pass's consumption
            # ratio (4 x chunks per wv tile) so neither starves the PE
            for c in range(1, KO):
                xload(c)
                if c % WBV == WBV - 2 and c // WBV + 1 < KO // WBV:
                    b = c // WBV + 1
                    nc.sync.dma_start(wvt[b],
                                      wvp[:, b * WBV:(b + 1) * WBV, :])
            tri = singles.tile([128, 128], MD)
            nc.sync.dma_start(tri, trip[:])
            zb = singles.tile([128, 1], F32)
            nc.vector.memset(zb, 0.0)
            csd = singles.tile([128, 2, S], MD)
            nc.sync.dma_start(csd[:, 0, :], csp[:, 0, :])
            nc.sync.dma_start(csd[:, 1, :], csp[:, 1, :])
            csA = csd[:, 0, :]
            csB = csd[:, 1, :]
            kT = [persist.tile([128, SA], MD, tag=f"kT{j}", name=f"kT{j}")
                  for j in range(KVL)]
            qT = [persist.tile([128, S], MD, tag=f"qT{h}", name=f"qT{h}")
                  for h in range(HL)]
            vv = persist.tile([128, KC, KVD], MD, tag="vv")
            sav = persist.tile([A, HL * HD], MD, tag="sav")
            nc.sync.dma_start(sav, savp[:])
            for j in range(KVL):
                nc.sync.dma_start(kT[j][:, S:SA], akp[j])
            oT = [persist.tile([128, S], MD, tag=f"oT{h}", name=f"oT{h}")
                  for h in range(HL)]
            woa = persist.tile([128, HL // 2, D], MD, tag="woa")

            # ---------------- V projection: accumulate in psum ------------
            # wv stays resident (adapter-V re-reads it); t-major emission
            # threads the psum->vv copies between banks so the V->K psum
            # handover doesn't wait on a burst of copies at the end
            with tc.tile_pool(name="psv", bufs=1, space="PSUM") as psv:
                vb = [psv.tile([128, KVD], F32, tag=f"vb{t}", name=f"vb{t}")
                      for t in range(KC)]
                for t in range(KC):
                    for c in range(KO):
                        nc.tensor.matmul(
                            vb[t], xsl(c)[:, t * 128:(t + 1) * 128],
                            wvt[c // WBV][:, c % WBV, :],
                            start=(c == 0), stop=(c == KO - 1))
                    # drain copies split across Act and DVE so the psum
                    # scope handover isn't serialized on one engine
                    if t % 2 == 1:
                        nc.scalar.copy(vv[:, t, :], vb[t])
                    else:
                        nc.vector.tensor_scalar_mul(vv[:, t, :], vb[t], 1.0)

            # ---------------- main pipelined scope ------------------------
            with tc.tile_pool(name="mp", bufs=1, space="PSUM") as mp:

                def emit_rope(ps_h, dst, hh):
                    sl = slice(hh * QB, (hh + 1) * QB)
                    rc = rpool.tile([128, QB], MD, tag="rc", bufs=2)
                    nc.scalar.copy(rc, ps_h)
                    rs = rpool.tile([128, QB], MD, tag="rs", bufs=2)
                    nc.sync.dma_start(rs[0:64, :], rc[64:128, :])
                    nc.sync.dma_start(rs[64:128, :], rc[0:64, :])
                    tm1 = rpool.tile([128, QB], MD, tag="tm1", bufs=2)
                    tm2 = rpool.tile([128, QB], MD, tag="tm2", bufs=2)
                    nc.vector.tensor_mul(tm1, rc, csA[:, sl])
                    nc.vector.tensor_mul(tm2, rs, csB[:, sl])
                    nc.vector.tensor_sub(dst[0:64, sl], tm1[0:64, :],
                                         tm2[0:64, :])
                    nc.vector.tensor_add(dst[64:128, sl], tm2[64:128, :],
                                         tm1[64:128, :])

                WBQ = 8

                def emit_head_proj(wparam, hidx, dst, first_wt=None):
                    ps_b = [mp.tile([128, QB], F32, tag="proj", bufs=2,
                                    name=f"pj{hidx}_{hh}")
                            for hh in range(NQH)]
                    wts = []
                    for b in range(KO // WBQ):
                        if b == 0 and first_wt is not None:
                            wts.append(first_wt)
                        else:
                            wt = wpool.tile([128, WBQ, HD], MD, tag="w",
                                            bufs=8)
                            nc.sync.dma_start(
                                wt, wparam[hidx, :, b * WBQ:(b + 1) * WBQ, :])
                            wts.append(wt)
                    for hh in range(NQH):
                        sl = slice(hh * QB, (hh + 1) * QB)
                        for c in range(KO):
                            xload(c)
                            st, sp = (c == 0), (c == KO - 1)
                            nc.tensor.matmul(ps_b[hh],
                                             wts[c // WBQ][:, c % WBQ, :],
                                             xsl(c)[:, sl],
                                             start=st, stop=sp)
                        emit_rope(ps_b[hh], dst, hh)

                def emit_attn_head(h, qb):
                    """Stage A: scores, exps, masking, denominator
                    accumulation, all-reduces, token PVs.  Returns a closure
                    (stage C) that finishes normalization one step later."""
                    j = h // nrep
                    qs, qe = qb * QB, (qb + 1) * QB
                    chunks = []
                    for kc in range(KC):
                        if kc * 128 >= qe:
                            break
                        q0 = max(qs, kc * 128)
                        chunks.append((kc, q0, qe - q0))
                    pairs = [chunks[i:i + 2]
                             for i in range(0, len(chunks), 2)]
                    entries = []
                    for pair in pairs:
                        scp = mp.tile([128, 2 * QB], F32, tag="scp",
                                      bufs=2)
                        pt = spool.tile([128, 2 * QB], MD, tag="pt",
                                        bufs=4)
                        for si, (kc, q0, N) in enumerate(pair):
                            nc.tensor.matmul(
                                scp[:, si * QB:si * QB + N],
                                kT[j][:, kc * 128:(kc + 1) * 128],
                                qT[h][:, q0:qe], start=True, stop=True)
                        NA = pair[0][2]
                        if len(pair) == 2 and NA == QB:
                            NB = pair[1][2]
                            nc.scalar.activation(pt[:, 0:QB + NB],
                                                 scp[:, 0:QB + NB], Exp,
                                                 bias=zb)
                        else:
                            nc.scalar.activation(pt[:, 0:NA],
                                                 scp[:, 0:NA], Exp,
                                                 bias=zb)
                            if len(pair) == 2:
                                NB = pair[1][2]
                                nc.scalar.activation(
                                    pt[:, QB:QB + NB],
                                    scp[:, QB:QB + NB], Exp, bias=zb)
                        for si, (kc, q0, N) in enumerate(pair):
                            if kc * 128 >= qs:
                                sd = si * QB
                                nc.vector.tensor_mul(
                                    pt[:, sd:sd + 128],
                                    pt[:, sd:sd + 128], tri)
                            entries.append((pt, si * QB, kc, q0, N))
                    # denominator: DVE-accumulate chunk probs, then
                    # partition-sum-broadcast on gpsimd
                    pts = spool.tile([128, QB], MD, tag="ptsum", bufs=2)
                    (p0, s0, _, _, _) = entries[0]
                    (p1, s1, _, _, N1) = entries[1]
                    c1 = QB - N1
                    nc.vector.tensor_add(pts[:, c1:QB],
                                         p0[:, s0 + c1:s0 + QB],
                                         p1[:, s1:s1 + N1])
                    if c1:
                        nc.vector.tensor_scalar_mul(
                            pts[:, 0:c1], p0[:, s0:s0 + c1], 1.0)
                    for (pe_, se_, _, _, Ne_) in entries[2:]:
                        ce = QB - Ne_
                        nc.vector.tensor_add(pts[:, ce:QB],
                                             pts[:, ce:QB],
                                             pe_[:, se_:se_ + Ne_])
                    dtb = spool.tile([128, QB], MD, tag="dtb", bufs=3)
                    nc.gpsimd.partition_all_reduce(dtb, pts, channels=128,
                                                   reduce_op=Radd)
                    ot = mp.tile([128, QB], F32, tag="ot", bufs=2)
                    for ki, (pt, sl_, kc, q0, N) in enumerate(entries):
                        nc.tensor.matmul(
                            ot[:, q0 - qs:QB],
                            vv[:, kc, j * HD:(j + 1) * HD],
                            pt[:, sl_:sl_ + N],
                            start=(ki == 0), stop=False)
                    # adapter block: separately-normalized, gate folded
                    # into sav; accumulates into the same psum bank
                    sca = mp.tile([A, QB], F32, tag="scp", bufs=2)
                    nc.tensor.matmul(sca, kT[j][:, S:SA],
                                     qT[h][:, qs:qe],
                                     start=True, stop=True)
                    pa = spool.tile([A, QB], MD, tag="pa", bufs=3)
                    nc.scalar.activation(pa, sca, Exp, bias=zb[0:A, :])
                    dab = spool.tile([A, QB], MD, tag="dab", bufs=3)
                    nc.gpsimd.partition_all_reduce(dab, pa, channels=A,
                                                   reduce_op=Radd)

                    def finish():
                        with nc.allow_low_precision(
                                reason="fp16 softmax scales, same rounding "
                                       "as every other fp16 operand"):
                            rca = spool.tile([A, QB], MD, tag="rca", bufs=2)
                            nc.vector.reciprocal(rca, dab)
                        # scale by dt/da: the shared psum gets divided by dt
                        # at the end, leaving the adapter block over da only
                        rca2 = spool.tile([A, QB], MD, tag="rca2", bufs=2)
                        nc.vector.tensor_mul(rca2, rca, dtb[0:A, :])
                        pan = spool.tile([A, QB], MD, tag="pan", bufs=2)
                        nc.vector.tensor_mul(pan, pa, rca2)
                        rct = spool.tile([128, QB], MD, tag="rct", bufs=2)
                        with nc.allow_low_precision(
                                reason="fp16 softmax scales, same rounding "
                                       "as every other fp16 operand"):
                            nc.vector.reciprocal(rct, dtb)
                        nc.tensor.matmul(ot,
                                         sav[0:A, h * HD:(h + 1) * HD],
                                         pan, start=False, stop=True)
                        nc.vector.tensor_mul(oT[h][:, qs:qe], ot, rct)

                    return finish

                # software pipeline: stage A of step k runs before the
                # normalization tail (stage C) of step k-1, so the next
                # block's mask/exp work leads the in-order DVE queue
                pending = [None]

                def attn_step(h, qb):
                    fin = emit_attn_head(h, qb)
                    if pending[0] is not None:
                        pending[0]()
                    pending[0] = fin

                emit_head_proj(wkp, 0, kT[0], first_wt=kwt0)
                emit_head_proj(wkp, 1, kT[1])
                # wo weights for heads 0..HL/2-1 (needed last, but queued
                # here so the stream never contends with projection weights)
                for hh in range(HL // 2):
                    for q4 in range(4):
                        nc.sync.dma_start(
                            woa[:, hh, q4 * (D // 4):(q4 + 1) * (D // 4)],
                            wop[:, hh, q4 * (D // 4):(q4 + 1) * (D // 4)])
                emit_head_proj(wqp, 0, qT[0])
                for h in range(1, HL):
                    emit_head_proj(wqp, h, qT[h])
                    attn_step(h - 1, 0)
                    attn_step(h - 1, 1)
                # wo weights for heads HL/2..HL-1 reuse dead x slots
                wob = [persist.tile([128, 1, D], MD, tag=f"x{NX - 4 + i}",
                                    name=f"wob{i}")
                       for i in range(HL // 2)]
                for i in range(HL // 2):
                    for q4 in range(4):
                        nc.sync.dma_start(
                            wob[i][:, 0, q4 * (D // 4):(q4 + 1) * (D // 4)],
                            wop[:, HL // 2 + i,
                                q4 * (D // 4):(q4 + 1) * (D // 4)])
                # last head: qb1 first and eagerly flushed, so the final
                # normalization tails overlap the output projection
                attn_step(HL - 1, 1)
                pending[0]()
                fin_last = emit_attn_head(HL - 1, 0)
                fin_last()

                # ---------------- output projection -----------------------
                # two half-head passes writing separate partial outputs
                # (host sums them): pass A only needs heads 0..HL/2-1, so
                # its matmuls keep the PE fed while the last heads'
                # softmax chains drain on DVE/Act/Pool
                def wsl(hh, n):
                    if hh < HL // 2:
                        return woa[:, hh, n * QB:(n + 1) * QB]
                    return wob[hh - HL // 2][:, 0, n * QB:(n + 1) * QB]

                if DBG:
                    nc.sync.dma_start(qTd[:], qT[0])
                    nc.sync.dma_start(kTd[:], kT[0])
                    nc.sync.dma_start(vvd[:], vv)
                    for h in range(HL):
                        nc.sync.dma_start(oTd[h], oT[h])

                NWO = D // QB
                NM = S // 128
                HH2 = HL // 2
                for half in range(2):
                    od = outp if half == 0 else outp2
                    # second pass: qb1-dependent rows first (they unblock
                    # as soon as the last head's qb1 tail finishes)
                    ms = (list(range(NM)) if half == 0 else
                          list(range(NM // 2, NM)) + list(range(NM // 2)))
                    for m in ms:
                        ob = None
                        for n in range(NWO):
                            pso = mp.tile([128, QB], F32, tag="proj",
                                          bufs=2)
                            for hi in range(HH2):
                                hh = half * HH2 + hi
                                nc.tensor.matmul(
                                    pso, oT[hh][:, m * 128:(m + 1) * 128],
                                    wsl(hh, n),
                                    start=(hi == 0), stop=(hi == HH2 - 1))
                            if ob is None:
                                ob = spool.tile([128, 2, QB], MD, tag="ob",
                                                bufs=6)
                            last = (half == 1 and n == NWO - 1
                                    and m == ms[-1])
                            nc.scalar.copy(ob[:, n % 2, :], pso)
                            if n % 2 == 1 and not last:
                                # one DMA per two tiles: halves the SP
                                # sequencer's per-DMA issue load
                                nc.sync.dma_start(
                                    od[m, :, (n - 1) * QB:(n + 1) * QB],
                                    ob[:, 0:2, :])
                                ob = None
                            elif last:
                                nc.sync.dma_start(
                                    od[m, :, (n - 1) * QB:n * QB],
                                    ob[:, 0, :])
                                nc.sync.dma_start(
                                    od[m, :, n * QB:(n + 1) * QB],
                                    ob[:, 1, :])

    nc.compile()
    nc.finalize()
    return nc


# --------------------------------------------------------------------------
# device program (legacy general-mask path)
# --------------------------------------------------------------------------

def build_program(KO, S, HL, KVL, causal, mm):
    """One NeuronCore's program.

    KO: D // 128 contraction chunks.  S: sequence length.  HL: q heads on
    this core.  KVL: kv heads on this core.  causal: hardwire causal
    masking (tri mask on diagonal chunks + chunk skipping); otherwise an
    additive mask [S, S] is an input.  mm: matmul operand dtype mode.
    """
    nc = bacc.Bacc(None, target_bir_lowering=False,
                   dynamic_dma_scratch_size=2048)
    MD = _md(mm)
    D = KO * 128
    QB = min(512, S)       # q column block (psum bank + fp32 moving max)
    NQH = S // QB
    KC = S // 128          # token key chunks
    SA = S + A
    nrep = HL // KVL

    xp = nc.declare_dram_parameter("xp", [128, KO, S], MD, isOutput=False)
    wqp = nc.declare_dram_parameter("wqp", [HL, 128, KO, HD], MD, isOutput=False)
    wkp = nc.declare_dram_parameter("wkp", [KVL, 128, KO, HD], MD, isOutput=False)
    wvp = nc.declare_dram_parameter("wvp", [128, KO, KVL * HD], MD, isOutput=False)
    wop = nc.declare_dram_parameter("wop", [128, HL, D], MD, isOutput=False)
    adp = nc.declare_dram_parameter("adp", [128, KO, A], MD, isOutput=False)
    csp = nc.declare_dram_parameter("csp", [128, 2, S], F32, isOutput=False)
    trip = nc.declare_dram_parameter("trip", [128, 128], MD, isOutput=False)
    gcp = nc.declare_dram_parameter("gcp", [1, HL * 128], MD, isOutput=False)
    if not causal:
        mtp = nc.declare_dram_parameter("mtp", [128, KC, S], F32, isOutput=False)
    outp = nc.declare_dram_parameter("out", [S // 128, 128, D], F32, isOutput=True)
    HSPLIT = 1
    if HSPLIT == 2:
        outp2 = nc.declare_dram_parameter("out2", [S // 128, 128, D], F32,
                                          isOutput=True)

    Exp = mybir.ActivationFunctionType.Exp

    with tile.TileContext(nc) as tc:
        with tc.tile_pool(name="singles", bufs=1) as singles, \
             tc.tile_pool(name="persist", bufs=1) as persist:
            tri = singles.tile([128, 128], MD)
            nc.sync.dma_start(tri, trip[:])

            # all-ones vectors: row 0 / column 127 of the tri mask
            ones_row = tri[0:1, :]
            ones_col = tri[:, 127:128]
            zb = singles.tile([128, 1], F32)
            nc.vector.memset(zb, 0.0)

            # resident x^T in XG-chunk tiles, DMA'd just-in-time from the
            # V-projection loop so the first matmuls start early
            XG = min(4, KO)
            NX = KO // XG
            xt = [persist.tile([128, XG, S], MD, tag=f"x{i}", name=f"x{i}")
                  for i in range(NX)]
            xt_loaded = [False] * NX

            def xload(i):
                if not xt_loaded[i]:
                    h = XG // 2 or 1
                    nc.sync.dma_start(xt[i][:, 0:h, :],
                                      xp[:, i * XG:i * XG + h, :])
                    if h < XG:
                        nc.sync.dma_start(xt[i][:, h:XG, :],
                                          xp[:, i * XG + h:(i + 1) * XG, :])
                    xt_loaded[i] = True

            def xsl(c):
                return xt[c // XG][:, c % XG, :]

            kT = [persist.tile([128, SA], MD, tag=f"kT{j}", name=f"kT{j}")
                  for j in range(KVL)]
            vv = persist.tile([128, KC + 1, KVL * HD], MD, tag="vv")
            qT = [persist.tile([128, S], MD, tag=f"qT{h}", name=f"qT{h}")
                  for h in range(HL)]

            # ---------------- phase 1: projections -----------------------
            with tc.tile_pool(name="wpool", bufs=3) as wpool, \
                 tc.tile_pool(name="rpool", bufs=2) as rpool, \
                 tc.tile_pool(name="cpool", bufs=1) as cpool:
                # csA: cos^T duplicated on both partition halves; csB: sin^T
                csd = cpool.tile([128, 2, S], F32)
                nc.sync.dma_start(csd, csp[:])
                csA = csd[:, 0, :]
                csB = csd[:, 1, :]
                adT = cpool.tile([128, KO, A], MD)
                nc.sync.dma_start(adT, adp[:])

                def emit_rope(ps_h, dst, hh):
                    # psum rows 0:64 = x0 (even pair elems), 64:128 = x1.
                    # dst[0:64] = x0*cos - x1*sin ; dst[64:128] = x0*sin + x1*cos
                    sl = slice(hh * QB, (hh + 1) * QB)
                    rc = rpool.tile([128, QB], F32, tag="rc", bufs=2)
                    nc.scalar.copy(rc, ps_h)        # frees the psum slot fast
                    rs = rpool.tile([128, QB], F32, tag="rs", bufs=2)
                    nc.sync.dma_start(rs[0:64, :], rc[64:128, :])
                    nc.sync.dma_start(rs[64:128, :], rc[0:64, :])
                    # tm1 = [x0*cos ; x1*cos], tm2 = [x1*sin ; x0*sin]
                    tm1 = rpool.tile([128, QB], F32, tag="tm1", bufs=1)
                    tm2 = rpool.tile([128, QB], F32, tag="tm2", bufs=1)
                    nc.vector.tensor_mul(tm1, rc, csA[:, sl])
                    nc.vector.tensor_mul(tm2, rs, csB[:, sl])
                    nc.vector.tensor_sub(dst[0:64, sl], tm1[0:64, :], tm2[0:64, :])
                    nc.vector.tensor_add(dst[64:128, sl], tm2[64:128, :],
                                         tm1[64:128, :])

                ps1cm = tc.tile_pool(name="ps1", bufs=1, space="PSUM")
                ps1 = ps1cm.__enter__()

                # ---- V projection (token-major): stream wv once; accumulate
                # the cross-block partials in an f32 SBUF tile so psum needs
                # only 2 banks.  Adapter V accumulates in its own bank.
                WBV = min(4, KO)
                NVB = KO // WBV
                vacc = cpool.tile([128, KC, KVL * HD], F32)
                pav = ps1.tile([A, KVL * HD], F32, tag="av")

                def emit_vblock(b):
                    wt = wpool.tile([128, WBV, KVL * HD], MD, tag="w")
                    nc.sync.dma_start(wt, wvp[:, b * WBV:(b + 1) * WBV, :])
                    for i in range(b * WBV // XG,
                                   (b * WBV + WBV - 1) // XG + 1):
                        xload(i)
                    for t in range(KC):
                        psv = ps1.tile([128, KVL * HD], F32, tag="vproj",
                                       bufs=2)
                        for ci in range(WBV):
                            c = b * WBV + ci
                            nc.tensor.matmul(
                                psv[:, :], xsl(c)[:, t * 128:(t + 1) * 128],
                                wt[:, ci, :],
                                start=(ci == 0), stop=(ci == WBV - 1))
                        if b == 0 and NVB > 1:
                            nc.scalar.copy(vacc[:, t, :], psv[:, :])
                        elif b < NVB - 1:
                            nc.vector.tensor_add(vacc[:, t, :], vacc[:, t, :],
                                                 psv[:, :])
                        elif NVB > 1:
                            nc.vector.tensor_add(vv[:, t, :], vacc[:, t, :],
                                                 psv[:, :])
                        else:
                            nc.scalar.copy(vv[:, t, :], psv[:, :])
                    for ci in range(WBV):
                        c = b * WBV + ci
                        nc.tensor.matmul(pav[:, :], adT[:, c, :], wt[:, ci, :],
                                         start=(c == 0), stop=(c == KO - 1))
                    if b == NVB - 1:
                        nc.scalar.copy(vv[0:A, KC, :], pav[:, :])

                WBQ = min(8, KO)

                def emit_khead(j):
                    psk = [ps1.tile([128, QB], F32, tag="proj", bufs=4,
                                    name=f"psk{hh}") for hh in range(NQH)]
                    pak = ps1.tile([128, A], F32, tag="ak")
                    for b in range(KO // WBQ):
                        wt = wpool.tile([128, WBQ, HD], MD, tag="w")
                        nc.sync.dma_start(wt, wkp[j, :, b * WBQ:(b + 1) * WBQ, :])
                        for i in range(b * WBQ // XG,
                                       (b * WBQ + WBQ - 1) // XG + 1):
                            xload(i)
                        for ci in range(WBQ):
                            c = b * WBQ + ci
                            st, sp = (c == 0), (c == KO - 1)
                            for hh in range(NQH):
                                sl = slice(hh * QB, (hh + 1) * QB)
                                nc.tensor.matmul(
                                    psk[hh][:, :], wt[:, ci, :], xsl(c)[:, sl],
                                    start=st, stop=sp)
                            nc.tensor.matmul(
                                pak[:, :], wt[:, ci, :], adT[:, c, :],
                                start=st, stop=sp)
                    for hh in range(NQH):
                        emit_rope(psk[hh], kT[j], hh)
                    nc.scalar.copy(kT[j][:, S:SA], pak[:, 0:A])

                def emit_qhead(h):
                    psq = [ps1.tile([128, QB], F32, tag="proj", bufs=4,
                                    name=f"psq{hh}") for hh in range(NQH)]
                    for b in range(KO // WBQ):
                        wt = wpool.tile([128, WBQ, HD], MD, tag="w")
                        nc.sync.dma_start(wt, wqp[h, :, b * WBQ:(b + 1) * WBQ, :])
                        for i in range(b * WBQ // XG,
                                       (b * WBQ + WBQ - 1) // XG + 1):
                            xload(i)
                        for ci in range(WBQ):
                            c = b * WBQ + ci
                            st, sp = (c == 0), (c == KO - 1)
                            for hh in range(NQH):
                                sl = slice(hh * QB, (hh + 1) * QB)
                                nc.tensor.matmul(
                                    psq[hh][:, :], wt[:, ci, :], xsl(c)[:, sl],
                                    start=st, stop=sp)
                    for hh in range(NQH):
                        emit_rope(psq[hh], qT[h], hh)

                # Interleave V blocks between K/Q head projections so the
                # DMA-heavy V stream overlaps compute-heavy head projections.
                kq = [("k", j) for j in range(KVL)] + \
                     [("q", h) for h in range(HL)]
                vb = list(range(NVB))
                seq = []
                while vb or kq:
                    if vb:
                        seq.append(("v", vb.pop(0)))
                    if kq:
                        seq.append(kq.pop(0))
                for kind, idx in seq:
                    if kind == "v":
                        emit_vblock(idx)
                    elif kind == "k":
                        emit_khead(idx)
                    else:
                        emit_qhead(idx)
                ps1cm.__exit__(None, None, None)

            # ---------------- phase 2: attention --------------------------
            # oT / wo-weights / general-mask reuse the dead x-tile slots
            HG = min(4, HL)
            oTt = [persist.tile([128, HG, S], MD,
                                tag=(f"x{i}" if i < NX else f"oT{i}"),
                                name=f"oTall{i}")
                   for i in range((HL + HG - 1) // HG)]

            def oT(h):
                return oTt[h // HG][:, h % HG, :]

            mt = None
            if not causal:
                mtt = [persist.tile([128, KC // 2, S], F32,
                                    tag=(f"x{4 + i}" if NX > 5 else f"mt{i}"),
                                    name=f"mt{i}")
                       for i in range(2)]
                nc.sync.dma_start(mtt[0], mtp[:, 0:KC // 2, :])
                nc.sync.dma_start(mtt[1], mtp[:, KC // 2:KC, :])

                def mtsl(kc):
                    return mtt[kc // (KC // 2)][:, kc % (KC // 2), :]
            with tc.tile_pool(name="spool", bufs=3) as spool, \
                 tc.tile_pool(name="ps2", bufs=1, space="PSUM") as ps2:
                gc = spool.tile([1, HL * 128], MD, tag="gc", bufs=1)
                nc.sync.dma_start(gc, gcp[:])
                for h in range(HL):
                    j = h // nrep
                    for qh in range(NQH):
                        qs, qe = qh * QB, (qh + 1) * QB
                        if causal:
                            kcs = [kc for kc in range(KC) if kc * 128 < qe]
                        else:
                            kcs = list(range(KC))
                        ot_ps = ps2.tile([128, QB], F32, tag="ot", bufs=2)
                        oa_ps = ps2.tile([128, QB], F32, tag="oa", bufs=1)
                        dt_ps = ps2.tile([1, QB], F32, tag="dt", bufs=1)
                        da_ps = ps2.tile([1, QB], F32, tag="da", bufs=1)
                        for ki, kc in enumerate(kcs):
                            q0 = max(qs, kc * 128) if causal else qs
                            N = qe - q0
                            st, sp = (ki == 0), (ki == len(kcs) - 1)
                            scp = ps2.tile([128, QB], F32, tag="scp", bufs=2)
                            nc.tensor.matmul(
                                scp[:, 0:N],
                                kT[j][:, kc * 128:(kc + 1) * 128],
                                qT[h][:, q0:qe], start=True, stop=True)
                            pt = spool.tile([128, QB], MD, tag="pt", bufs=4)
                            if causal:
                                nc.scalar.activation(pt[:, 0:N], scp[:, 0:N],
                                                     Exp, bias=zb)
                                if kc * 128 >= qs:  # diagonal chunk
                                    nc.vector.tensor_mul(
                                        pt[:, 0:128], pt[:, 0:128], tri)
                            else:
                                sadd = spool.tile([128, QB], F32, tag="sadd",
                                                  bufs=2)
                                nc.vector.tensor_add(
                                    sadd[:, 0:N], scp[:, 0:N],
                                    mtsl(kc)[:, q0:qe])
                                nc.scalar.activation(pt[:, 0:N], sadd[:, 0:N],
                                                     Exp, bias=zb)
                            nc.tensor.matmul(
                                ot_ps[:, q0 - qs:QB],
                                vv[:, kc, j * HD:(j + 1) * HD],
                                pt[:, 0:N], start=st, stop=sp)
                            nc.tensor.matmul(
                                dt_ps[0:1, q0 - qs:QB], ones_col[:, 0:1],
                                pt[:, 0:N], start=st, stop=sp)
                        # adapter block
                        sca = ps2.tile([128, QB], F32, tag="scp", bufs=2)
                        nc.tensor.matmul(sca[0:A, :], kT[j][:, S:SA],
                                         qT[h][:, qs:qe], start=True, stop=True)
                        pa = spool.tile([128, QB], MD, tag="pt", bufs=4)
                        nc.scalar.activation(pa[0:A, :], sca[0:A, :], Exp,
                                             bias=zb[0:A, :])
                        nc.tensor.matmul(oa_ps[:, :],
                                         vv[0:A, KC, j * HD:(j + 1) * HD],
                                         pa[0:A, :], start=True, stop=True)
                        nc.tensor.matmul(da_ps[0:1, :], ones_col[0:A, 0:1],
                                         pa[0:A, :], start=True, stop=True)
                        # normalization factors (per-q scalars), f32r direct
                        rt = spool.tile([1, QB], MD, tag="rt", bufs=1)
                        ra = spool.tile([1, QB], MD, tag="ra", bufs=1)
                        with nc.allow_low_precision(
                                reason="f32r softmax scales, rounded like "
                                       "every other matmul operand"):
                            nc.vector.reciprocal(rt, dt_ps[0:1, :])
                            nc.vector.reciprocal(ra, da_ps[0:1, :])
                        # broadcast across partitions via rank-1 matmul;
                        # tanh(gate_h) is folded into the adapter lhsT (gc)
                        rp1 = ps2.tile([128, QB], F32, tag="rp", bufs=1)
                        nc.tensor.matmul(rp1, ones_row[0:1, :], rt[0:1, :],
                                         start=True, stop=True)
                        rtb = spool.tile([128, QB], F32, tag="rtb", bufs=1)
                        nc.scalar.copy(rtb, rp1)
                        rp2 = ps2.tile([128, QB], F32, tag="rp", bufs=1)
                        nc.tensor.matmul(rp2, gc[0:1, h * 128:(h + 1) * 128],
                                         ra[0:1, :], start=True, stop=True)
                        rab = spool.tile([128, QB], F32, tag="rab", bufs=1)
                        nc.scalar.copy(rab, rp2)
                        # oT = ot/denom_t + tanh(g)*oa/denom_a  (write-once)
                        tq1 = spool.tile([128, QB], F32, tag="tq1", bufs=1)
                        nc.vector.tensor_mul(tq1, ot_ps[:, :], rtb)
                        tq2 = spool.tile([128, QB], F32, tag="tq2", bufs=1)
                        nc.vector.tensor_mul(tq2, oa_ps[:, :], rab)
                        nc.vector.tensor_add(oT(h)[:, qs:qe], tq1, tq2)

            # ---------------- phase 3: output projection ------------------
            # Split into two half-head passes writing separate partial
            # outputs (host sums them): the first pass only needs heads
            # 0..HL/2-1 and overlaps the tail of the attention phase.
            with tc.tile_pool(name="wopool", bufs=2) as wopool, \
                 tc.tile_pool(name="obpool", bufs=3) as obpool, \
                 tc.tile_pool(name="ps3", bufs=4, space="PSUM") as ps3:
                NB = D // 512
                HH = HL // HSPLIT
                # deep prefetch of wo weight tiles into freed x slots
                slots = [i for i in range(2, NX)
                         if causal or i not in (4, 5)] or [None]
                for half in range(HSPLIT):
                    od = outp if half == 0 else outp2
                    h0 = half * HH
                    for n in range(NB):
                        wi = slots[(half * NB + n) % len(slots)]
                        if wi is not None and NX > wi:
                            won = persist.tile([128, HH, 512], MD,
                                               tag=f"x{wi}",
                                               name=f"won{half}_{n}")
                        else:
                            won = wopool.tile([128, HH, 512], MD, tag="won")
                        for wpi in range(0, HH, 2):
                            nc.sync.dma_start(
                                won[:, wpi:wpi + 2, :],
                                wop[:, h0 + wpi:h0 + wpi + 2,
                                    n * 512:(n + 1) * 512])
                        for m in range(S // 128):
                            pso = ps3.tile([128, 512], F32, tag="wo", bufs=6)
                            for hh2 in range(HH):
                                nc.tensor.matmul(
                                    pso,
                                    oT(h0 + hh2)[:, m * 128:(m + 1) * 128],
                                    won[:, hh2, :],
                                    start=(hh2 == 0), stop=(hh2 == HH - 1))
                            ob = obpool.tile([128, 512], F32, tag="ob")
                            nc.scalar.copy(ob, pso)
                            nc.sync.dma_start(
                                od[m, :, n * 512:(n + 1) * 512], ob)

    nc.compile()
    nc.finalize()
    return nc


def get_program(KO, S, HL, KVL, causal, mm):
    key = (KO, S, HL, KVL, causal, mm)
    if key not in _PROG_CACHE:
        if causal:
            _PROG_CACHE[key] = build_program_v2(KO, S, HL, KVL, mm)
        else:
            _PROG_CACHE[key] = build_program(KO, S, HL, KVL, causal, mm)
    return _PROG_CACHE[key]


# --------------------------------------------------------------------------
# host-side sharding / layout prep
# --------------------------------------------------------------------------

_EVEN_FIRST = np.concatenate([np.arange(0, HD, 2), np.arange(1, HD, 2)])


def is_causal_mask(mask):
    S = mask.shape[-1]
    m = np.asarray(mask).reshape(S, S)
    iu = np.triu_indices(S, 1)
    il = np.tril_indices(S)
    return bool(np.all(m[il] == 0.0) and np.all(m[iu] <= -1e8))


def _np_md(mm):
    if mm == "bf16":
        import ml_dtypes
        return ml_dtypes.bfloat16
    if mm == "fp16":
        return np.float16
    return np.float32


def prep_core_inputs_v2(core, G, x, wq, wk, wv, wo, adapter, gate,
                        freqs_cos, freqs_sin, mm=None):
    """v2 layout: fp16 tensors, tanh(gate) table instead of gcp, single
    fp16 partial output per core."""
    mm = MM_MODE if mm is None else mm
    B, S, D = x.shape
    H = gate.shape[1]
    hd = wq.shape[1] // H
    KV = wk.shape[1] // hd
    KO = D // 128
    HL, KVL = H // G, KV // G
    b, g = core // G, core % G
    hsl = slice(g * HL, (g + 1) * HL)
    ksl = slice(g * KVL, (g + 1) * KVL)
    idx = _EVEN_FIRST
    f32 = np.float32
    md = _np_md(mm)

    def c(a, dt=None):
        return np.ascontiguousarray(a, dtype=dt if dt is not None else md)

    xp = c(x[b].T.reshape(KO, 128, S).transpose(1, 0, 2))
    wq4 = wq.reshape(D, H, hd)[:, hsl][:, :, idx] * np.float32(1.0 / np.sqrt(hd))
    wqp = c(wq4.reshape(KO, 128, HL, hd).transpose(2, 1, 0, 3))
    wk4 = wk.reshape(D, KV, hd)[:, ksl][:, :, idx]
    wkp = c(wk4.reshape(KO, 128, KVL, hd).transpose(2, 1, 0, 3))
    wv4 = wv.reshape(D, KV, hd)[:, ksl]
    wvp = c(wv4.reshape(KO, 128, KVL * hd).transpose(1, 0, 2))
    wos = wo[g * HL * hd:(g + 1) * HL * hd]
    wop = c(wos.reshape(HL, hd, D).transpose(1, 0, 2))
    ad = np.asarray(adapter[0], dtype=f32)
    ak = np.einsum("ad,dje->jea", ad, wk4.astype(f32))       # [KVL, hd, A]
    av = np.einsum("ad,dje->aje", ad, wv4.astype(f32))       # [A, KVL, hd]
    gth = np.tanh(np.asarray(gate[0, hsl, 0, 0], dtype=np.float64))
    nrep = (wq.shape[1] // hd) // KV * 1  # q heads per kv head (global)
    nrep = HL // KVL
    savp = np.empty((A, HL * hd), f32)
    for h in range(HL):
        savp[:, h * hd:(h + 1) * hd] = av[:, h // nrep, :] * gth[h]
    ct = np.asarray(freqs_cos, dtype=f32).T
    st = np.asarray(freqs_sin, dtype=f32).T
    csp = np.empty((128, 2, S), f32)
    csp[0:64, 0] = ct
    csp[64:128, 0] = ct
    csp[0:64, 1] = st
    csp[64:128, 1] = st
    tri = c(np.triu(np.ones((128, 128), dtype=f32)))
    return {"xp": xp, "wqp": wqp, "wkp": wkp, "wvp": wvp, "wop": wop,
            "csp": c(csp), "trip": tri, "akp": c(ak), "savp": c(savp)}


def prep_core_inputs(core, G, x, wq, wk, wv, wo, adapter, gate,
                     freqs_cos, freqs_sin, mask, causal, mm=None):
    """Build the input dict for one core = (batch b, head-group g)."""
    mm = MM_MODE if mm is None else mm
    B, S, D = x.shape
    H = gate.shape[1]
    hd = wq.shape[1] // H
    KV = wk.shape[1] // hd
    KO = D // 128
    KC = S // 128
    HL, KVL = H // G, KV // G
    b, g = core // G, core % G
    hsl = slice(g * HL, (g + 1) * HL)
    ksl = slice(g * KVL, (g + 1) * KVL)
    idx = _EVEN_FIRST
    f32 = np.float32
    md = _np_md(mm)

    def c(a, dt=None):
        return np.ascontiguousarray(a, dtype=dt if dt is not None else md)

    xp = c(x[b].T.reshape(KO, 128, S).transpose(1, 0, 2))
    wq4 = wq.reshape(D, H, hd)[:, hsl][:, :, idx] * np.float32(1.0 / np.sqrt(hd))
    wqp = c(wq4.reshape(KO, 128, HL, hd).transpose(2, 1, 0, 3))
    wk4 = wk.reshape(D, KV, hd)[:, ksl][:, :, idx]
    wkp = c(wk4.reshape(KO, 128, KVL, hd).transpose(2, 1, 0, 3))
    wv4 = wv.reshape(D, KV, hd)[:, ksl]
    wvp = c(wv4.reshape(KO, 128, KVL * hd).transpose(1, 0, 2))
    wos = wo[g * HL * hd:(g + 1) * HL * hd]
    wop = c(wos.reshape(HL, hd, D).transpose(1, 0, 2))
    adp = c(adapter[0].T.reshape(KO, 128, A).transpose(1, 0, 2))
    # cos^T / sin^T, each duplicated across both partition halves
    ct = np.asarray(freqs_cos, dtype=f32).T      # [64, S]
    st = np.asarray(freqs_sin, dtype=f32).T
    csp = np.empty((128, 2, S), f32)
    csp[0:64, 0] = ct
    csp[64:128, 0] = ct
    csp[0:64, 1] = st
    csp[64:128, 1] = st
    tri = c(np.triu(np.ones((128, 128), dtype=f32)))
    gth = np.tanh(np.asarray(gate[0, hsl, 0, 0], dtype=np.float64)).astype(f32)
    gcp = c(np.repeat(gth, 128).reshape(1, HL * 128))
    inp = {"xp": xp, "wqp": wqp, "wkp": wkp, "wvp": wvp, "wop": wop,
           "adp": adp, "csp": csp, "trip": tri, "gcp": gcp}
    if not causal:
        mt = np.asarray(mask).reshape(S, S).T  # [keys, q]
        inp["mtp"] = c(mt.reshape(KC, 128, S).transpose(1, 0, 2), f32)
    return inp


# --------------------------------------------------------------------------
# entry point
# --------------------------------------------------------------------------

def kernel(x, wq, wk, wv, wo, adapter, gate, freqs_cos, freqs_sin, mask,
           _trace=False):
    x, wq, wk, wv, wo, adapter, gate, freqs_cos, freqs_sin, mask = (
        np.asarray(a) for a in
        (x, wq, wk, wv, wo, adapter, gate, freqs_cos, freqs_sin, mask))
    B, S, D = x.shape
    H = gate.shape[1]
    hd = wq.shape[1] // H
    KV = wk.shape[1] // hd
    G = 8 // B                      # head groups per batch over 8 cores
    HL, KVL = H // G, KV // G
    KO = D // 128

    causal = is_causal_mask(mask)
    nc = get_program(KO, S, HL, KVL, causal, MM_MODE)

    if causal:
        in_maps = [prep_core_inputs_v2(core, G, x, wq, wk, wv, wo, adapter,
                                       gate, freqs_cos, freqs_sin)
                   for core in range(8)]
    else:
        in_maps = [prep_core_inputs(core, G, x, wq, wk, wv, wo, adapter, gate,
                                    freqs_cos, freqs_sin, mask, causal)
                   for core in range(8)]
    res = run_bass_kernel_spmd(nc, in_maps, core_ids=list(range(8)),
                               trace=_trace)
    out = np.zeros((B, S, D), np.float32)
    for core in range(8):
        b = core // G
        r = res.results[core]
        out[b] += r["out"].astype(np.float32).reshape(S, D)
        if "out2" in r:
            out[b] += r["out2"].astype(np.float32).reshape(S, D)
    if _trace:
        kernel._last_result = res
    return out

